# revision 1
# baseline (speedup 1.0000x reference)
"""MetaPathAggregator kernel V2 — dual-path gather (Pool ap_gather + DMA gather).

Math (linear collapse): out[t] = T0[i0]+T1[i1]+T2[i2]+T3[i3] with
T_k = feat_k @ M_k, M_k built from the weight matrices.

Per core (TOK=16384 tokens):
- Pool path (A_TOK tokens): tables stored feature-major as PACKED bf16 pairs in
  f32 channels ([128 ch, 1024 rows] f32; ch c of half h = feats (2c,2c+1) of
  slot). One gpsimd.ap_gather per slot-PAIR (each 16-partition core group uses
  its own index stream): 2 instructions / chunk-slot-pair.
  Reduction: S1 = gA + gB (DVE bf16 add), then PE matmul with stacked identity
  (psum[c] = S1[c] + S1[64+c]), ACT psum->bf16 copy into staging, 1 store/chunk.
- DMA path (B_TOK tokens): SBUF-source transpose-mode dma_gather from a
  combined staged table [128, 32, 128] bf16 (row i at partition i%128, rank
  i//128; slot order g1,g2,mi,dr). 2 instructions x 2 slots. DVE adds, 1 store.
- DMA instruction count kept minimal (HWDGE fixed cost ~625ns each, serialized).
- Outputs bf16 feature-major; host transposes + upcasts (layout only).
"""

import numpy as np
import ml_dtypes

P = 128
F = 128
H = 128
HH = 64
R = 1024                 # padded table rows (indices < 1000)
NT = R // P              # 8 row-tiles per table
N_CORES = 8
B_PAIRS = 1024
BAG = 128
TOK = B_PAIRS * BAG // N_CORES   # 16384
A_TOK = 12800                    # pool-path tokens
B_TOK = TOK - A_TOK              # dma-path tokens
CHUNKS_A = (2048, 2048, 2048, 2048, 2048, 2048, 512)   # sums to A_TOK
assert sum(CHUNKS_A) == A_TOK
CH_D = 896                       # dma-path add chunk
NCH_D = B_TOK // CH_D
RED = 512                        # psum reduce chunk (bf16 cols)

# merged idx tensor column offsets (int16 columns)
IDX_A0 = 0
IDX_B0 = A_TOK // 16
IDX_D1 = 2 * (A_TOK // 16)
IDX_D2 = IDX_D1 + 2 * B_TOK // 16
IDX_COLS = IDX_D2 + 2 * B_TOK // 16

_CACHE = {}


def _build_module(do_pool=True, do_dma=True):
    import concourse.bacc as bacc
    import concourse.mybir as mybir
    import concourse.tile as tile
    from concourse.masks import make_identity

    f32 = mybir.dt.float32
    bf16 = mybir.dt.bfloat16
    i16 = mybir.dt.int16
    Copy = mybir.ActivationFunctionType.Copy

    nc = bacc.Bacc("TRN2", dynamic_dma_scratch_size=32768)

    feats_in = {
        "mi": nc.dram_tensor("feat_mi", [R, F], f32, kind="ExternalInput"),
        "ge": nc.dram_tensor("feat_ge", [R, F], f32, kind="ExternalInput"),
        "dr": nc.dram_tensor("feat_dr", [R, F], f32, kind="ExternalInput"),
    }
    w2_in = nc.dram_tensor("w2", [2 * H, F], f32, kind="ExternalInput")    # wdd, wdg
    wh_in = nc.dram_tensor("wh", [HH, 2 * F], f32, kind="ExternalInput")   # [wdrug|wdis]
    idx_in = nc.dram_tensor("idx", [P, IDX_COLS], i16, kind="ExternalInput")
    out_p = nc.dram_tensor("out_p", [P, A_TOK], bf16, kind="ExternalOutput")
    out_d = nc.dram_tensor("out_d", [P, B_TOK], bf16, kind="ExternalOutput")

    with tile.TileContext(nc) as tc:
        with (
            tc.tile_pool(name="const", bufs=1) as cpool,
            tc.tile_pool(name="prep", bufs=3) as ppool,
            tc.tile_pool(name="main", bufs=2) as mpool,
        ):
            import contextlib
            prep_psum = contextlib.ExitStack()
            trpool = prep_psum.enter_context(
                tc.tile_pool(name="trps", bufs=2, space="PSUM"))
            stpool = prep_psum.enter_context(
                tc.tile_pool(name="stps", bufs=2, space="PSUM"))
            pkpool = prep_psum.enter_context(
                tc.tile_pool(name="pkps", bufs=1, space="PSUM"))
            # ---------------- loads (few, large) ----------------
            wh = cpool.tile([HH, 2 * F], f32, tag="wh")
            nc.sync.dma_start(wh[:], wh_in[:, :])
            w2 = cpool.tile([P, 2, F], f32, tag="w2")
            nc.sync.dma_start(w2[:], w2_in[:, :].rearrange("(g p) f -> p g f", p=P))
            featf = {}
            for name in ("mi", "ge", "dr"):
                ft = cpool.tile([P, NT, F], f32, tag=f"featf_{name}")
                nc.sync.dma_start(
                    ft[:], feats_in[name][:, :].rearrange("(n p) f -> p n f", p=P))
                featf[name] = ft
            idx = cpool.tile([P, IDX_COLS], i16, tag="idx")
            nc.sync.dma_start(idx[:], idx_in[:, :])

            wdd_t = w2[:, 0, :]
            wdg_t = w2[:, 1, :]
            wdrug_t = wh[:, 0:F]
            wdis_t = wh[:, F:2 * F]

            # ---------------- constants ----------------
            ident = cpool.tile([P, P], f32, tag="ident")
            make_identity(nc, ident[:])
            # stacked identity [128, 64] bf16: I2[k, m] = (k % 64 == m)
            i2 = cpool.tile([P, HH], bf16, tag="i2")
            nc.vector.tensor_copy(out=i2[0:HH, :], in_=ident[0:HH, 0:HH])
            nc.sync.dma_start(i2[HH:P, :], i2[0:HH, :])

            # ---------------- weight math (f32) ----------------
            cd_ps = stpool.tile([P, 4 * P], f32, tag="stps", name="cd_ps")
            nc.tensor.transpose(out=cd_ps[:, 0:HH], in_=wdrug_t, identity=ident[:HH, :HH])
            nc.tensor.transpose(out=cd_ps[:, P:P + HH], in_=wdis_t, identity=ident[:HH, :HH])
            c_s = cpool.tile([F, HH], f32, tag="c_s")
            nc.vector.tensor_copy(out=c_s[:], in_=cd_ps[:, 0:HH])
            d_s = cpool.tile([F, HH], f32, tag="d_s")
            nc.vector.tensor_copy(out=d_s[:], in_=cd_ps[:, P:P + HH])
            ab_ps = stpool.tile([P, 4 * P], f32, tag="stps", name="ab_ps")
            nc.tensor.matmul(out=ab_ps[:, 0:HH], lhsT=wdd_t, rhs=d_s[:], start=True, stop=True)
            nc.tensor.matmul(out=ab_ps[:, P:P + HH], lhsT=wdg_t, rhs=c_s[:], start=True, stop=True)
            a_s = cpool.tile([F, HH], f32, tag="a_s")
            nc.scalar.activation(out=a_s[:], in_=ab_ps[:, 0:HH], func=Copy)
            b_s = cpool.tile([F, HH], f32, tag="b_s")
            nc.scalar.activation(out=b_s[:], in_=ab_ps[:, P:P + HH], func=Copy)

            # ---------------- M matrices (bf16) ----------------
            pieces = {0: (c_s, 0.5, a_s, 0.125), 1: (c_s, 0.25, a_s, 0.125),
                      2: (b_s, 0.125, d_s, 0.25), 3: (b_s, 0.125, d_s, 0.5)}
            m_full, m_ev, m_od = {}, {}, {}
            for k in range(4):
                lo, slo, hi, shi = pieces[k]
                mk = cpool.tile([F, H], bf16, tag=f"m{k}")
                nc.scalar.activation(out=mk[:, :HH], in_=lo[:], func=Copy,
                                     scale=float(slo))
                nc.scalar.activation(out=mk[:, HH:], in_=hi[:], func=Copy,
                                     scale=float(shi))
                m_full[k] = mk
                lo3 = lo[:].rearrange("p (r two) -> p r two", two=2)
                hi3 = hi[:].rearrange("p (r two) -> p r two", two=2)
                ev = cpool.tile([F, HH], bf16, tag=f"mev{k}")
                nc.vector.tensor_scalar_mul(ev[:, 0:32], lo3[:, :, 0], slo)
                nc.vector.tensor_scalar_mul(ev[:, 32:64], hi3[:, :, 0], shi)
                od = cpool.tile([F, HH], bf16, tag=f"mod{k}")
                nc.vector.tensor_scalar_mul(od[:, 0:32], lo3[:, :, 1], slo)
                nc.vector.tensor_scalar_mul(od[:, 32:64], hi3[:, :, 1], shi)
                m_ev[k] = ev
                m_od[k] = od

            # ---------------- table transforms (4-row-tile batches) ----------
            # staged slot order (mi, g1, g2, dr) -> rank bases
            # gather1 = ranks 0:16 (mi, g1) ready mid-prep; gather2 = 16:32
            rank_base = {0: 0, 1: 8, 2: 16, 3: 24}
            dstag = cpool.tile([P, 32, P], bf16, tag="dstag")
            a_pack = cpool.tile([P, R], f32, tag="apack")
            b_pack = cpool.tile([P, R], f32, tag="bpack")
            packs = {0: (a_pack, 0), 1: (a_pack, 1), 2: (b_pack, 0), 3: (b_pack, 1)}

            slot_feat = {0: "mi", 1: "ge", 2: "ge", 3: "dr"}
            W = 4 * P  # batch width (4 row-tiles)
            fts_t = {}
            alt = [0]

            def get_fts(name, b):
                if (name, b) in fts_t:
                    return fts_t[(name, b)]
                ts0 = 4 * b
                tr = trpool.tile([P, W], f32, tag="ftps", name=f"tr_{name}_{b}")
                for i in range(4):
                    nc.tensor.transpose(
                        out=tr[:, i * P:(i + 1) * P],
                        in_=featf[name][:, ts0 + i, :], identity=ident[:])
                fts = ppool.tile([P, W], bf16, tag=f"fts_{name}{b}",
                                 name=f"fts_{name}_{b}")
                if alt[0] % 2 == 0:
                    nc.vector.tensor_copy(out=fts[:], in_=tr[:])
                else:
                    nc.scalar.activation(out=fts[:], in_=tr[:], func=Copy)
                alt[0] += 1
                fts_t[(name, b)] = fts
                return fts

            def do_packed(k, b):
                fts = get_fts(slot_feat[k], b)
                dest, half = packs[k]
                h0 = half * HH
                pe_ps = pkpool.tile([P, W], f32, tag="pkev", name=f"pe_{k}_{b}")
                po_ps = pkpool.tile([P, W], f32, tag="pkod", name=f"po_{k}_{b}")
                for i in range(4):
                    cs = slice(i * P, (i + 1) * P)
                    nc.tensor.matmul(out=pe_ps[h0:h0 + HH, cs],
                                     lhsT=m_ev[k][:], rhs=fts[:, cs],
                                     start=True, stop=True)
                    nc.tensor.matmul(out=po_ps[h0:h0 + HH, cs],
                                     lhsT=m_od[k][:], rhs=fts[:, cs],
                                     start=True, stop=True)
                dv = dest[:].bitcast(bf16).rearrange("p (r two) -> p r two", two=2)
                rs = slice(4 * b * P, (4 * b + 4) * P)
                nc.vector.tensor_copy(out=dv[h0:h0 + HH, rs, 0],
                                      in_=pe_ps[h0:h0 + HH, :])
                nc.scalar.activation(out=dv[h0:h0 + HH, rs, 1],
                                     in_=po_ps[h0:h0 + HH, :], func=Copy)

            def do_staged(k, b):
                fts = get_fts(slot_feat[k], b)
                ts0 = 4 * b
                st_ps = stpool.tile([P, W], f32, tag="stps", name=f"st_{k}_{b}")
                for i in range(4):
                    cs = slice(i * P, (i + 1) * P)
                    nc.tensor.matmul(out=st_ps[:, cs], lhsT=fts[:, cs],
                                     rhs=m_full[k][:], start=True, stop=True)
                if (k + b) % 2 == 0:
                    nc.vector.tensor_copy(
                        out=dstag[:, rank_base[k] + ts0:rank_base[k] + ts0 + 4, :],
                        in_=st_ps[:])
                else:
                    nc.scalar.activation(
                        out=dstag[:, rank_base[k] + ts0:rank_base[k] + ts0 + 4, :],
                        in_=st_ps[:], func=Copy)

            # pass 1: a_pack (slots 0, 1) -- unblocks the pool-path gathers
            for k in (0, 1):
                for b in range(2):
                    do_packed(k, b)
            # pass 2: b_pack (slots 2, 3), then staged tables in gather order
            for k in (2, 3):
                for b in range(2):
                    do_packed(k, b)
            for k in (0, 1, 2, 3):
                for b in range(2):
                    do_staged(k, b)

            # close prep PSUM pools; main loop gets a deep reduce ring
            prep_psum.close()
            rd_psum = contextlib.ExitStack()
            rdpool = rd_psum.enter_context(
                tc.tile_pool(name="rdps", bufs=6, space="PSUM"))

            from concourse.tile_rust import add_dep_helper

            # ---------------- pool-path chunks ----------------
            s1_adds = []
            ap_instrs = []

            def pool_chunk(c, off, size):
                cols = slice(IDX_A0 + off // 16, IDX_A0 + (off + size) // 16)
                colsb = slice(IDX_B0 + off // 16, IDX_B0 + (off + size) // 16)
                CM = max(CHUNKS_A)
                ga_f = mpool.tile([P, CM], f32, tag="ga", name=f"ga{c}", bufs=4)
                ga = ga_f[:, :size]
                ap_instrs.append(nc.gpsimd.ap_gather(
                    ga, a_pack[:], idx[:, cols], P, R, 1, size))
                gb_f = mpool.tile([P, CM], f32, tag="gb", name=f"gb{c}", bufs=4)
                gb = gb_f[:, :size]
                ap_instrs.append(nc.gpsimd.ap_gather(
                    gb, b_pack[:], idx[:, colsb], P, R, 1, size))
                ga_bf = ga.bitcast(bf16)
                gb_bf = gb.bitcast(bf16)
                # staging [128, size]: even RED blocks in parts 0:64, odd 64:128
                stg_f = mpool.tile([P, CM], bf16, tag="stg", name=f"stg{c}")
                stg = stg_f[:, :size]
                nred = 2 * size // RED
                for j in range(0, nred, 2):
                    # two reduce results share one [128, RED] psum (halves);
                    # PSUM accumulates ga + gb (all four slot-tables)
                    ps = rdpool.tile([P, RED], f32, tag="rd", name=f"rd{c}_{j}")
                    jj = j // 2
                    cs = slice(jj * RED, (jj + 1) * RED)
                    for h, jx in ((0, j), (1, j + 1)):
                        hs = slice(h * HH, (h + 1) * HH)
                        sl = slice(jx * RED, (jx + 1) * RED)
                        nc.tensor.matmul(out=ps[hs, :], lhsT=i2[:],
                                         rhs=ga_bf[:, sl],
                                         start=True, stop=False)
                        nc.tensor.matmul(out=ps[hs, :], lhsT=i2[:],
                                         rhs=gb_bf[:, sl],
                                         start=False, stop=True)
                    if jj % 2 == 0:
                        nc.scalar.activation(out=stg[0:HH, cs], in_=ps[0:HH, :],
                                             func=Copy)
                        nc.scalar.activation(out=stg[HH:P, cs], in_=ps[HH:P, :],
                                             func=Copy)
                    else:
                        nc.vector.tensor_copy(out=stg[0:HH, cs], in_=ps[0:HH, :])
                        nc.vector.tensor_copy(out=stg[HH:P, cs], in_=ps[HH:P, :])
                nc.sync.dma_start(out_p[:, off:off + size], stg)

            # first pool chunk emitted before the desc-gens; the desc-gens get
            # explicit deps on the first aps so the scheduler cannot place the
            # (late-ready) desc-gens at the head of the Pool stream.
            off = 0
            if do_pool:
                pool_chunk(0, 0, CHUNKS_A[0])
                off = CHUNKS_A[0]

            g_d1 = cpool.tile([P, 1, 2 * B_TOK], bf16, tag="gd1")
            g_d2 = cpool.tile([P, 1, 2 * B_TOK], bf16, tag="gd2")
            if do_dma:
                gi1 = nc.gpsimd.dma_gather(
                    g_d1[:], dstag[:, 0:16, :],
                    idx[:, IDX_D1:IDX_D1 + 2 * B_TOK // 16],
                    2 * B_TOK, 2 * B_TOK, P,
                    transpose=True, single_packet=False,
                    sbuf_tokens_per_rank=128, sbuf_free_dim_per_rank=256,
                    sbuf_free_dim_pad_per_rank=0, sbuf_byte_offset=0,
                )
                gi2 = nc.gpsimd.dma_gather(
                    g_d2[:], dstag[:, 16:32, :],
                    idx[:, IDX_D2:IDX_D2 + 2 * B_TOK // 16],
                    2 * B_TOK, 2 * B_TOK, P,
                    transpose=True, single_packet=False,
                    sbuf_tokens_per_rank=128, sbuf_free_dim_per_rank=256,
                    sbuf_free_dim_pad_per_rank=0, sbuf_byte_offset=0,
                )
                if do_pool and ap_instrs:
                    add_dep_helper(gi1.ins, ap_instrs[0].ins, reason="pool order")
                    add_dep_helper(gi2.ins, ap_instrs[1].ins, reason="pool order")

            if do_pool:
                for c in range(1, len(CHUNKS_A) - 2):
                    pool_chunk(c, off, CHUNKS_A[c])
                    off += CHUNKS_A[c]

            # ---------------- DMA-path adds + one store ----------------
            # emitted before the last pool chunks so the out_d store is not
            # queued behind the final out_p stores on SP
            if do_dma:
                sd = cpool.tile([P, B_TOK], bf16, tag="sd")
                for c in range(NCH_D):
                    sl = slice(c * CH_D, (c + 1) * CH_D)
                    sl2 = slice(B_TOK + c * CH_D, B_TOK + (c + 1) * CH_D)
                    t01 = mpool.tile([P, CH_D], bf16, tag="t01", name=f"t01_{c}")
                    i1a = nc.vector.tensor_add(t01[:], g_d1[:, 0, sl], g_d1[:, 0, sl2])
                    t23 = mpool.tile([P, CH_D], bf16, tag="t23", name=f"t23_{c}")
                    i2a = nc.vector.tensor_add(t23[:], g_d2[:, 0, sl], g_d2[:, 0, sl2])
                    nc.vector.tensor_add(sd[:, sl], t01[:], t23[:])
                    if do_pool and ap_instrs:
                        # keep DVE from head-of-line blocking behind the big
                        # gather transfers: spread these behind pool chunks
                        anchor = ap_instrs[min(2 * c + 1, len(ap_instrs) - 1)]
                        add_dep_helper(i1a.ins, anchor.ins, reason="dve order")
                        add_dep_helper(i2a.ins, anchor.ins, reason="dve order")
                    if c == NCH_D // 2 - 1:
                        nc.sync.dma_start(out_d[:, :B_TOK // 2],
                                          sd[:, :B_TOK // 2])
                nc.sync.dma_start(out_d[:, B_TOK // 2:], sd[:, B_TOK // 2:])

            if do_pool:
                for c in range(len(CHUNKS_A) - 2, len(CHUNKS_A)):
                    pool_chunk(c, off, CHUNKS_A[c])
                    off += CHUNKS_A[c]

            rd_psum.close()

    nc.compile()
    return nc


def _wrap16(v):
    """token j -> [j % 16, j // 16] layout."""
    return np.ascontiguousarray(v.reshape(-1, 16).T)


def _prep_inputs(feat_miRNA, feat_gene, feat_drug, W_drug_disease, W_disease_drug,
                 W_drug, W_dis, mp_ins):
    def pad_rows(a):
        a = np.ascontiguousarray(np.asarray(a, dtype=np.float32))
        if a.shape[0] >= R:
            return np.ascontiguousarray(a[:R])
        out = np.zeros((R, a.shape[1]), dtype=np.float32)
        out[: a.shape[0]] = a
        return out

    f_mi = pad_rows(feat_miRNA)
    f_ge = pad_rows(feat_gene)
    f_dr = pad_rows(feat_drug)
    w2 = np.concatenate([
        np.asarray(W_drug_disease, np.float32),
        np.asarray(W_disease_drug, np.float32)], axis=0)
    w2 = np.ascontiguousarray(w2)
    wh = np.concatenate([
        np.asarray(W_drug, np.float32), np.asarray(W_dis, np.float32)], axis=1)
    wh = np.ascontiguousarray(wh)

    mp = np.asarray(mp_ins)
    assert mp.shape == (B_PAIRS, BAG, 4), mp.shape

    in_maps = []
    for core in range(N_CORES):
        mp_core = mp[core * (B_PAIRS // N_CORES):(core + 1) * (B_PAIRS // N_CORES)]
        mp_core = mp_core.reshape(TOK, 4).astype(np.int16)
        i0, i1, i2, i3 = (mp_core[:, k] for k in range(4))
        idx = np.empty((P, IDX_COLS), dtype=np.int16)
        # pool path: first A_TOK tokens; A = (i0 -> cores 0-3, i1 -> cores 4-7)
        idx[0:64, IDX_A0:IDX_B0] = np.tile(_wrap16(i0[:A_TOK]), (4, 1))
        idx[64:128, IDX_A0:IDX_B0] = np.tile(_wrap16(i1[:A_TOK]), (4, 1))
        idx[0:64, IDX_B0:IDX_D1] = np.tile(_wrap16(i2[:A_TOK]), (4, 1))
        idx[64:128, IDX_B0:IDX_D1] = np.tile(_wrap16(i3[:A_TOK]), (4, 1))
        # dma path: last B_TOK tokens; staged slot order (mi, g1, g2, dr)
        d1 = np.concatenate([i0[A_TOK:], R + i1[A_TOK:]]).astype(np.int16)
        d2 = np.concatenate([i2[A_TOK:], R + i3[A_TOK:]]).astype(np.int16)
        idx[:, IDX_D1:IDX_D2] = np.tile(_wrap16(d1), (8, 1))
        idx[:, IDX_D2:IDX_COLS] = np.tile(_wrap16(d2), (8, 1))
        in_maps.append({"feat_mi": f_mi, "feat_ge": f_ge, "feat_dr": f_dr,
                        "w2": w2, "wh": wh, "idx": idx})
    return in_maps


def _assemble(results):
    outs = []
    for r in results:
        op = np.asarray(r["out_p"]).astype(np.float32)      # [128, A_TOK]
        od = np.asarray(r["out_d"]).astype(np.float32)      # [128, B_TOK]
        # per chunk: op[h*64+cf, off + jj*RED + 2*s2 + l] = feat(2cf+l) of
        # token off + jj*512 + h*256 + s2
        parts = []
        off = 0
        for L in CHUNKS_A:
            a = op[:, off:off + L].reshape(2, HH, L // RED, RED // 2, 2)
            a = a.transpose(2, 0, 3, 1, 4)                  # [jj, h, s2, cf, l]
            parts.append(a.reshape(L, H))
            off += L
        pool = np.concatenate(parts, axis=0)
        outs.append(np.concatenate([pool, od.T], axis=0))
    return np.concatenate(outs, axis=0).reshape(B_PAIRS, BAG, H)


def _numpy_fallback(feat_miRNA, feat_gene, feat_drug, W_drug_disease,
                    W_disease_drug, W_drug, W_dis, mp_ins):
    mi = np.asarray(feat_miRNA, np.float32)[mp_ins[:, :, 0]]
    g1 = np.asarray(feat_gene, np.float32)[mp_ins[:, :, 1]]
    g2 = np.asarray(feat_gene, np.float32)[mp_ins[:, :, 2]]
    dr = np.asarray(feat_drug, np.float32)[mp_ins[:, :, 3]]
    wdd = np.asarray(W_drug_disease, np.float32)
    wdg = np.asarray(W_disease_drug, np.float32)
    wdrug = np.asarray(W_drug, np.float32)
    wdis = np.asarray(W_dis, np.float32)
    dis = ((((mi + g1) * 0.5) @ wdd.T + g2) * 0.5 + dr) * 0.5
    drug = ((((dr + g2) * 0.5) @ wdg.T + g1) * 0.5 + mi) * 0.5
    return np.concatenate([drug @ wdrug.T, dis @ wdis.T], axis=2)


def kernel(**inputs):
    mp = np.asarray(inputs["mp_ins"])
    if mp.max() >= R or mp.min() < 0:
        return _numpy_fallback(**inputs)

    from concourse.bass_utils import run_bass_kernel_spmd

    if "nc" not in _CACHE:
        _CACHE["nc"] = _build_module()
    nc = _CACHE["nc"]

    in_maps = _prep_inputs(**inputs)
    res = run_bass_kernel_spmd(nc, in_maps, core_ids=list(range(N_CORES)))
    return _assemble(res.results)


if __name__ == "__main__":
    import reference

    inputs = {k: np.asarray(v) for k, v in reference.setup_inputs().items()}
    expected = np.asarray(reference.reference(**inputs))
    actual = kernel(**inputs)
    rel = np.linalg.norm(actual - expected) / np.linalg.norm(expected)
    print("Relative error:", rel)

    from concourse.timeline_sim import TimelineSim
    print("TimelineSim:", TimelineSim(_CACHE["nc"], trace=False).simulate(), "ns")



# revision 5
# speedup vs baseline: 1.1190x; 1.1190x over previous
"""MetaPathAggregator kernel V4 — pair-packed transformed tables (Pool path)
+ raw HBM dma_gather (DMA path).

Math (linear collapse): out[t] = sum_k feat_k[i_k[t]] @ M_k with
  M0 = [0.5*Wdrug^T | 0.125*Wdd^T Wdis^T]   (mi)
  M1 = [0.25*Wdrug^T | 0.125*Wdd^T Wdis^T]  (g1)
  M2 = [0.125*Wdg^T Wdrug^T | 0.25*Wdis^T]  (g2)
  M3 = [0.125*Wdg^T Wdrug^T | 0.5*Wdis^T]   (dr)

Per core (TOK=16384):
- A-path (A_TOK tokens, chunks of 2048): transformed tables T_k = feat_k @ M_k
  stored as PACKED bf16 pairs in f32 containers: a_pack parts 0:64 = T0 pairs
  (2c,2c+1), parts 64:128 = T1 pairs; b_pack = T2|T3.  One ap_gather per
  container per chunk (idx streams i0/i1 resp i2/i3 per partition half).
  Reduce: PSUM accumulate with stacked identity (psum[c] = S[c]+S[64+c]) over
  ga+gb, two 256-token blocks stacked in psum partition halves; ONE full
  [128,512] psum->bf16 copy per block-pair (ACT/DVE alternating); 1 store.
- D-path (D_TOK tokens, 2 halves): dma_gather raw bf16 rows straight from HBM
  feat_all [3072,128] (row offset selects table), transpose mode ->
  g[f-part, slot*DH+tok].  Reduce: 4 matmuls lhsT=M_k per 512-token psum
  chunk; copy; store.  No staged table, no prep dependency.
- Outputs bf16; host transposes + upcasts (layout only).
"""

import numpy as np
import ml_dtypes

P = 128
F = 128
H = 128
HH = 64
R = 1024                 # padded table rows (indices < 1000)
NT = R // P              # 8 row-tiles per table
N_CORES = 8
B_PAIRS = 1024
BAG = 128
TOK = B_PAIRS * BAG // N_CORES   # 16384
CH_A = 2048
N_CH_A = 6
A_TOK = CH_A * N_CH_A            # 12288
D_TOK = TOK - A_TOK              # 4096
DH = D_TOK // 2                  # tokens per dma_gather (2048)
CH_D = 512                       # D-path psum chunk
RED = 512                        # A-path psum reduce chunk (bf16 cols)

# idx tile columns (int16, [128, IDX_COLS])
IDX_A0 = 0                       # ga streams (i0 | i1)
IDX_B0 = A_TOK // 16             # gb streams (i2 | i3)
IDX_D1 = 2 * (A_TOK // 16)
DC = 4 * DH // 16                # 512 cols per dma_gather
IDX_D2 = IDX_D1 + DC
IDX_COLS = IDX_D2 + DC

_CACHE = {}


def _build_module():
    import contextlib

    import concourse.bacc as bacc
    import concourse.mybir as mybir
    import concourse.tile as tile
    from concourse.masks import make_identity

    f32 = mybir.dt.float32
    bf16 = mybir.dt.bfloat16
    i16 = mybir.dt.int16
    Copy = mybir.ActivationFunctionType.Copy

    nc = bacc.Bacc("TRN2", dynamic_dma_scratch_size=32768)

    feat_in = nc.dram_tensor("feat_all", [3 * R, F], bf16, kind="ExternalInput")
    wh_in = nc.dram_tensor("wh", [HH, 2 * F], f32, kind="ExternalInput")   # [Wdrug|Wdis]
    w2_in = nc.dram_tensor("w2", [2 * H, F], f32, kind="ExternalInput")    # [Wdd;Wdg]
    idx_in = nc.dram_tensor("idx", [P, IDX_COLS], i16, kind="ExternalInput")
    out_p = nc.dram_tensor("out_p", [P, A_TOK], bf16, kind="ExternalOutput")
    out_d = nc.dram_tensor("out_d", [P, D_TOK], bf16, kind="ExternalOutput")

    with tile.TileContext(nc) as tc:
        with (
            tc.tile_pool(name="const", bufs=1) as cpool,
            tc.tile_pool(name="prep", bufs=3) as ppool,
            tc.tile_pool(name="main", bufs=2) as mpool,
        ):
            prep_ps = contextlib.ExitStack()
            wpool = prep_ps.enter_context(
                tc.tile_pool(name="wps", bufs=1, space="PSUM"))
            trpool = prep_ps.enter_context(
                tc.tile_pool(name="trps", bufs=2, space="PSUM"))
            pkpool = prep_ps.enter_context(
                tc.tile_pool(name="pkps", bufs=2, space="PSUM"))

            # ---------------- loads ----------------
            idx = cpool.tile([P, IDX_COLS], i16, tag="idx")
            nc.sync.dma_start(idx[:], idx_in[:, :])
            wh = cpool.tile([HH, 2 * F], f32, tag="wh")
            nc.sync.dma_start(wh[:], wh_in[:, :])
            w2 = cpool.tile([P, 2, F], f32, tag="w2")
            nc.sync.dma_start(w2[:], w2_in[:, :].rearrange("(g p) f -> p g f", p=P))
            fa = cpool.tile([P, 3 * NT, F], bf16, tag="fa")
            nc.sync.dma_start(fa[:], feat_in[:, :].rearrange("(n p) f -> p n f", p=P))

            # ---------------- D-path gathers (raw rows straight from HBM) ---
            g_d = []
            for half in range(2):
                g = cpool.tile([P, 1, 4 * DH], bf16, tag=f"gd{half}")
                o = (IDX_D1, IDX_D2)[half]
                nc.gpsimd.dma_gather(
                    g[:], feat_in[:, :], idx[:, o:o + DC],
                    4 * DH, 4 * DH, F,
                    transpose=True, single_packet=False,
                )
                g_d.append(g)

            # ---------------- constants ----------------
            ident = cpool.tile([P, P], f32, tag="ident")
            make_identity(nc, ident[:])
            identb = cpool.tile([P, P], bf16, tag="identb")
            nc.vector.tensor_copy(out=identb[:], in_=ident[:])
            # stacked identity [128, 64] bf16: I2[k, m] = (k % 64 == m)
            i2 = cpool.tile([P, HH], bf16, tag="i2")
            nc.vector.tensor_copy(out=i2[0:HH, :], in_=ident[0:HH, 0:HH])
            nc.sync.dma_start(i2[HH:P, :], i2[0:HH, :])

            # ---------------- weight math (f32) ----------------
            # wps: C = Wdrug^T at cols 0:64, D = Wdis^T at cols 128:192
            wps = wpool.tile([P, 2 * P], f32, tag="wps", name="wps")
            nc.tensor.transpose(out=wps[:, 0:HH], in_=wh[:, 0:F],
                                identity=ident[0:HH, 0:HH])
            nc.tensor.transpose(out=wps[:, P:P + HH], in_=wh[:, F:2 * F],
                                identity=ident[0:HH, 0:HH])
            c_s = cpool.tile([F, HH], f32, tag="c_s")
            nc.scalar.activation(out=c_s[:], in_=wps[:, 0:HH], func=Copy)
            d_s = cpool.tile([F, HH], f32, tag="d_s")
            nc.scalar.activation(out=d_s[:], in_=wps[:, P:P + HH], func=Copy)
            # A = Wdd^T Wdis^T, B = Wdg^T Wdrug^T
            wps2 = wpool.tile([P, 2 * P], f32, tag="wps2", name="wps2")
            nc.tensor.matmul(out=wps2[:, 0:HH], lhsT=w2[:, 0, :], rhs=d_s[:],
                             start=True, stop=True)
            nc.tensor.matmul(out=wps2[:, P:P + HH], lhsT=w2[:, 1, :], rhs=c_s[:],
                             start=True, stop=True)
            a_s = cpool.tile([F, HH], f32, tag="a_s")
            nc.scalar.activation(out=a_s[:], in_=wps2[:, 0:HH], func=Copy)
            b_s = cpool.tile([F, HH], f32, tag="b_s")
            nc.scalar.activation(out=b_s[:], in_=wps2[:, P:P + HH], func=Copy)

            # M_k full (bf16, for D-path) and ev/od column splits (for A-path)
            pieces = {0: (c_s, 0.5, a_s, 0.125), 1: (c_s, 0.25, a_s, 0.125),
                      2: (b_s, 0.125, d_s, 0.25), 3: (b_s, 0.125, d_s, 0.5)}
            m_full, m_ev, m_od = {}, {}, {}
            for k in range(4):
                lo, slo, hi, shi = pieces[k]
                mk = cpool.tile([F, H], bf16, tag=f"m{k}")
                nc.scalar.activation(out=mk[:, :HH], in_=lo[:], func=Copy,
                                     scale=float(slo))
                nc.scalar.activation(out=mk[:, HH:], in_=hi[:], func=Copy,
                                     scale=float(shi))
                m_full[k] = mk
                lo3 = lo[:].rearrange("p (r two) -> p r two", two=2)
                hi3 = hi[:].rearrange("p (r two) -> p r two", two=2)
                ev = cpool.tile([F, HH], bf16, tag=f"mev{k}")
                nc.vector.tensor_scalar_mul(ev[:, 0:32], lo3[:, :, 0], slo)
                nc.vector.tensor_scalar_mul(ev[:, 32:64], hi3[:, :, 0], shi)
                od = cpool.tile([F, HH], bf16, tag=f"mod{k}")
                nc.vector.tensor_scalar_mul(od[:, 0:32], lo3[:, :, 1], slo)
                nc.vector.tensor_scalar_mul(od[:, 32:64], hi3[:, :, 1], shi)
                m_ev[k] = ev
                m_od[k] = od

            # ---------------- packed table build ----------------
            # fts: transposed bf16 feats per 4-row-tile batch
            W = 4 * P
            fts_t = {}
            alt = [0]
            base = {"mi": 0, "ge": NT, "dr": 2 * NT}

            def get_fts(name, b):
                if (name, b) in fts_t:
                    return fts_t[(name, b)]
                tr = trpool.tile([P, W], bf16, tag="ftps", name=f"tr_{name}_{b}")
                for i in range(4):
                    nc.tensor.transpose(
                        out=tr[:, i * P:(i + 1) * P],
                        in_=fa[:, base[name] + 4 * b + i, :], identity=identb[:])
                fts = ppool.tile([P, W], bf16, tag=f"fts_{name}{b}",
                                 name=f"fts_{name}_{b}")
                if alt[0] % 2 == 0:
                    nc.vector.tensor_copy(out=fts[:], in_=tr[:])
                else:
                    nc.scalar.activation(out=fts[:], in_=tr[:], func=Copy)
                alt[0] += 1
                fts_t[(name, b)] = fts
                return fts

            # containers: a_pack = T0|T1 pairs, b_pack = T2|T3
            a_pack = cpool.tile([P, R], f32, tag="apack")
            b_pack = cpool.tile([P, R], f32, tag="bpack")
            packs = {"a": (a_pack, 0, 1, "mi", "ge"),
                     "b": (b_pack, 2, 3, "ge", "dr")}

            def build_pack(which, b):
                dest, klo, khi, nlo, nhi = packs[which]
                flo = get_fts(nlo, b)
                fhi = get_fts(nhi, b)
                pe_ps = pkpool.tile([P, W], f32, tag="pkev", name=f"pe_{which}_{b}")
                po_ps = pkpool.tile([P, W], f32, tag="pkod", name=f"po_{which}_{b}")
                for i in range(4):
                    cs = slice(i * P, (i + 1) * P)
                    nc.tensor.matmul(out=pe_ps[0:HH, cs], lhsT=m_ev[klo][:],
                                     rhs=flo[:, cs], start=True, stop=True)
                    nc.tensor.matmul(out=pe_ps[HH:P, cs], lhsT=m_ev[khi][:],
                                     rhs=fhi[:, cs], start=True, stop=True)
                    nc.tensor.matmul(out=po_ps[0:HH, cs], lhsT=m_od[klo][:],
                                     rhs=flo[:, cs], start=True, stop=True)
                    nc.tensor.matmul(out=po_ps[HH:P, cs], lhsT=m_od[khi][:],
                                     rhs=fhi[:, cs], start=True, stop=True)
                dv = dest[:].bitcast(bf16).rearrange("p (r two) -> p r two", two=2)
                rs = slice(b * W, (b + 1) * W)
                nc.scalar.activation(out=dv[:, rs, 0], in_=pe_ps[:], func=Copy)
                nc.vector.tensor_copy(out=dv[:, rs, 1], in_=po_ps[:])

            for b in range(2):
                build_pack("a", b)
            for b in range(2):
                build_pack("b", b)

            prep_ps.close()
            main_ps = contextlib.ExitStack()
            apool = main_ps.enter_context(
                tc.tile_pool(name="aps", bufs=6, space="PSUM"))
            dpool = main_ps.enter_context(
                tc.tile_pool(name="dps", bufs=2, space="PSUM"))

            # ---------------- main loops ----------------
            calt = [0]

            def a_chunk(c):
                off = c * CH_A
                cols = slice(IDX_A0 + off // 16, IDX_A0 + (off + CH_A) // 16)
                colsb = slice(IDX_B0 + off // 16, IDX_B0 + (off + CH_A) // 16)
                ga = mpool.tile([P, CH_A], f32, tag="ga", name=f"ga{c}", bufs=3)
                nc.gpsimd.ap_gather(ga[:], a_pack[:], idx[:, cols], P, R, 1, CH_A)
                gb = mpool.tile([P, CH_A], f32, tag="gb", name=f"gb{c}", bufs=3)
                nc.gpsimd.ap_gather(gb[:], b_pack[:], idx[:, colsb], P, R, 1, CH_A)
                ga_bf = ga[:].bitcast(bf16)
                gb_bf = gb[:].bitcast(bf16)
                stg = mpool.tile([P, CH_A], bf16, tag="stg", name=f"stg{c}", bufs=3)
                nred = 2 * CH_A // RED
                for j in range(0, nred, 2):
                    ps = apool.tile([P, RED], f32, tag="aps", name=f"aps{c}_{j}")
                    jj = j // 2
                    cs = slice(jj * RED, (jj + 1) * RED)
                    for h, jx in ((0, j), (1, j + 1)):
                        hs = slice(h * HH, (h + 1) * HH)
                        sl = slice(jx * RED, (jx + 1) * RED)
                        nc.tensor.matmul(out=ps[hs, :], lhsT=i2[:],
                                         rhs=ga_bf[:, sl], start=True, stop=False)
                        nc.tensor.matmul(out=ps[hs, :], lhsT=i2[:],
                                         rhs=gb_bf[:, sl], start=False, stop=True)
                    if calt[0] % 2 == 0:
                        nc.scalar.activation(out=stg[:, cs], in_=ps[:], func=Copy)
                    else:
                        nc.vector.tensor_copy(out=stg[:, cs], in_=ps[:])
                    calt[0] += 1
                nc.sync.dma_start(out_p[:, off:off + CH_A], stg[:])

            def d_half(half):
                g = g_d[half]
                stg = mpool.tile([P, DH], bf16, tag="stgd", name=f"stgd{half}")
                for j in range(DH // CH_D):
                    js = slice(j * CH_D, (j + 1) * CH_D)
                    ps = dpool.tile([P, CH_D], f32, tag="dps", name=f"dps{half}_{j}")
                    for k in range(4):
                        sl = slice(k * DH + j * CH_D, k * DH + (j + 1) * CH_D)
                        nc.tensor.matmul(out=ps[:], lhsT=m_full[k][:],
                                         rhs=g[:, 0, sl], start=(k == 0),
                                         stop=(k == 3))
                    if calt[0] % 2 == 0:
                        nc.scalar.activation(out=stg[:, js], in_=ps[:], func=Copy)
                    else:
                        nc.vector.tensor_copy(out=stg[:, js], in_=ps[:])
                    calt[0] += 1
                nc.sync.dma_start(out_d[:, half * DH:(half + 1) * DH], stg[:])

            a_chunk(0)
            a_chunk(1)
            a_chunk(2)
            d_half(0)
            a_chunk(3)
            a_chunk(4)
            d_half(1)
            a_chunk(5)

            main_ps.close()

    nc.compile()
    return nc


def _wrap16(v):
    """token j -> [j % 16, j // 16] layout."""
    return np.ascontiguousarray(v.reshape(-1, 16).T)


def _prep_inputs(feat_miRNA, feat_gene, feat_drug, W_drug_disease, W_disease_drug,
                 W_drug, W_dis, mp_ins):
    def pad_rows(a):
        a = np.asarray(a, dtype=np.float32)
        out = np.zeros((R, a.shape[1]), dtype=np.float32)
        out[: min(R, a.shape[0])] = a[:R]
        return out

    feat_all = np.concatenate(
        [pad_rows(feat_miRNA), pad_rows(feat_gene), pad_rows(feat_drug)]
    ).astype(ml_dtypes.bfloat16)
    wh = np.ascontiguousarray(np.concatenate(
        [np.asarray(W_drug, np.float32), np.asarray(W_dis, np.float32)], axis=1))
    w2 = np.ascontiguousarray(np.concatenate(
        [np.asarray(W_drug_disease, np.float32),
         np.asarray(W_disease_drug, np.float32)], axis=0))

    mp = np.asarray(mp_ins)
    assert mp.shape == (B_PAIRS, BAG, 4), mp.shape

    in_maps = []
    for core in range(N_CORES):
        mp_core = mp[core * (B_PAIRS // N_CORES):(core + 1) * (B_PAIRS // N_CORES)]
        mp_core = mp_core.reshape(TOK, 4).astype(np.int16)
        i0, i1, i2, i3 = (mp_core[:, k] for k in range(4))
        idx = np.empty((P, IDX_COLS), dtype=np.int16)
        idx[0:HH, IDX_A0:IDX_B0] = np.tile(_wrap16(i0[:A_TOK]), (4, 1))
        idx[HH:P, IDX_A0:IDX_B0] = np.tile(_wrap16(i1[:A_TOK]), (4, 1))
        idx[0:HH, IDX_B0:IDX_D1] = np.tile(_wrap16(i2[:A_TOK]), (4, 1))
        idx[HH:P, IDX_B0:IDX_D1] = np.tile(_wrap16(i3[:A_TOK]), (4, 1))
        for half in range(2):
            t = slice(A_TOK + half * DH, A_TOK + (half + 1) * DH)
            d = np.concatenate(
                [i0[t], R + i1[t], R + i2[t], 2 * R + i3[t]]).astype(np.int16)
            o = (IDX_D1, IDX_D2)[half]
            idx[:, o:o + DC] = np.tile(_wrap16(d), (8, 1))
        in_maps.append({"feat_all": feat_all, "wh": wh, "w2": w2, "idx": idx})
    return in_maps


def _assemble(results):
    outs = []
    for r in results:
        op = np.asarray(r["out_p"]).astype(np.float32)      # [128, A_TOK]
        od = np.asarray(r["out_d"]).astype(np.float32)      # [128, D_TOK]
        # op[h*64+c, ch*2048 + jj*512 + 2*s + l] = feat(2c+l) of token
        #   ch*2048 + jj*512 + h*256 + s
        a = op.reshape(2, HH, N_CH_A, 4, RED // 2, 2)
        a = a.transpose(2, 3, 0, 4, 1, 5).reshape(A_TOK, H)
        outs.append(np.concatenate([a, od.T], axis=0))
    return np.concatenate(outs, axis=0).reshape(B_PAIRS, BAG, H)


def _numpy_fallback(feat_miRNA, feat_gene, feat_drug, W_drug_disease,
                    W_disease_drug, W_drug, W_dis, mp_ins):
    mi = np.asarray(feat_miRNA, np.float32)[mp_ins[:, :, 0]]
    g1 = np.asarray(feat_gene, np.float32)[mp_ins[:, :, 1]]
    g2 = np.asarray(feat_gene, np.float32)[mp_ins[:, :, 2]]
    dr = np.asarray(feat_drug, np.float32)[mp_ins[:, :, 3]]
    wdd = np.asarray(W_drug_disease, np.float32)
    wdg = np.asarray(W_disease_drug, np.float32)
    wdrug = np.asarray(W_drug, np.float32)
    wdis = np.asarray(W_dis, np.float32)
    dis = ((((mi + g1) * 0.5) @ wdd.T + g2) * 0.5 + dr) * 0.5
    drug = ((((dr + g2) * 0.5) @ wdg.T + g1) * 0.5 + mi) * 0.5
    return np.concatenate([drug @ wdrug.T, dis @ wdis.T], axis=2)


def kernel(**inputs):
    mp = np.asarray(inputs["mp_ins"])
    if mp.max() >= R or mp.min() < 0:
        return _numpy_fallback(**inputs)

    from concourse.bass_utils import run_bass_kernel_spmd

    if "nc" not in _CACHE:
        _CACHE["nc"] = _build_module()
    nc = _CACHE["nc"]

    in_maps = _prep_inputs(**inputs)
    res = run_bass_kernel_spmd(nc, in_maps, core_ids=list(range(N_CORES)))
    return _assemble(res.results)


if __name__ == "__main__":
    import reference

    inputs = {k: np.asarray(v) for k, v in reference.setup_inputs().items()}
    expected = np.asarray(reference.reference(**inputs))
    actual = kernel(**inputs)
    rel = np.linalg.norm(actual - expected) / np.linalg.norm(expected)
    print("Relative error:", rel)

    from concourse.timeline_sim import TimelineSim
    print("TimelineSim:", TimelineSim(_CACHE["nc"], trace=False).simulate(), "ns")
